# revision 30
# baseline (speedup 1.0000x reference)
"""Trainium2 Bass kernel for nn_DIMESDenseEncoder (GNN message passing).

Self-contained: hardcodes B=16, N=200, U=64, L=3, 8 cores, batch-sharded
(2 graphs per core). Dense edge layout [src*200+dst] with masked diagonal;
feature-major SBUF layout: partition = graph_half*64 + feature.
BatchNorm stats are exact: per-core partial sums, cross-core AllReduce,
analytic diagonal correction. Edge tensors (w, z) live in SBUF as bf16.

v2: bf16 embed broadcast (one matmul per chunk, K=2), manual 8-bank PSUM
rotation with 4-bank grouped ACT copies (+S accumulator), SS via DVE
scalar_tensor_tensor at 2x, split AllReduce (edge stats hidden behind
pooling, node stats behind apply), bf16 x-matmuls, last-layer final output
split as e1^T.w_old + e1^T.lr with term1 staged via DRAM during the AR gap.
"""
import os
import numpy as np
import ml_dtypes
import concourse.bass as bass
import concourse.tile as tile
from concourse import bacc, mybir
from concourse.bass_utils import run_bass_kernel_spmd

F32, BF16 = mybir.dt.float32, mybir.dt.bfloat16
F16 = mybir.dt.float16
AF = mybir.ActivationFunctionType
OP = mybir.AluOpType

B, N, U, L = 16, 200, 64, 3
EPS = 1e-5
NCORES = 8
BC = B // NCORES            # graphs per core
NCOL = N * N                # dense edge cols per graph-half = 40000
CNT_E = B * N * (N - 1)     # global real-edge count
CNT_V = B * N               # global node count

ZCH = 400                   # z chunk = 2 sources, one PSUM bank
ZG = 4 * ZCH                # z copy group = 4 banks = 1600 cols
NZG = NCOL // ZG            # 25 groups per layer
PCH_S = 10                  # pooling chunk sources
PCH = PCH_S * N             # pooling chunk cols = 2000
ACH = 2000                  # apply chunk cols (10 sources)
OCH = 500                   # final-output sub-chunk
ECH = 500                   # embed matmul sub-chunk (one bank)
EG = 4 * ECH                # embed group cols = 2000
BANK = 512                  # PSUM bank stride in f32 cols

# ---- consts_f32 column layout (host-packed) ----
C_VB0 = 0        # v_lin0_b stacked
C_EB0 = 1        # e_lin0_b stacked
C_EL1B = 2       # e_lin1_b at p {0,1,32,33,64,65,96,97}
C_B1 = 3         # v_b1[i] stacked (3 cols)
C_B2 = 6
C_B3 = 9         # v_b3[i]+e_b[i] stacked
C_B4 = 12
C_EG = 15        # e_bn_g[i] p0-63
C_EB = 18
C_VG = 21
C_VB = 24
C_FOLD = 27      # P_fold [128,64]
C_EXP = 91       # E_exp [64,128]
C_XW = 219       # v_lin0_w rows 0-1 / 64-65
CF = C_XW + 64

# ---- consts_bf16 columns ----
# 128*i : e_w[i] blockdiag [128,128]; 384:386 e_lin1 blockdiag;
# 386:514 I128; 514:642 embed stationary (rows 0,1,32,33);
# 642: v_wk[i] stacked-two-copies, 12 blocks of 64
C_EMB = 514
C_VWB = 642
CB = C_VWB + 12 * 64

_CACHE = {}


def _build_consts(inp):
    f = np.zeros((128, CF), np.float32)
    bfc = np.zeros((128, CB), np.float32)

    def stack(v):
        return np.concatenate([v, v]).astype(np.float32)

    f[:, C_VB0] = stack(inp['v_lin0_b'])
    f[:, C_EB0] = stack(inp['e_lin0_b'])
    for _q in (0, 32, 64, 96):
        f[_q, C_EL1B] = f[_q + 1, C_EL1B] = inp['e_lin1_b'][0]
    for i in range(L):
        f[:, C_B1 + i] = stack(inp['v_b1'][i])
        f[:, C_B2 + i] = stack(inp['v_b2'][i])
        f[:, C_B3 + i] = stack(inp['v_b3'][i] + inp['e_b'][i])
        f[:, C_B4 + i] = stack(inp['v_b4'][i])
        f[:64, C_EG + i] = inp['e_bn_g'][i]
        f[:64, C_EB + i] = inp['e_bn_b'][i]
        f[:64, C_VG + i] = inp['v_bn_g'][i]
        f[:64, C_VB + i] = inp['v_bn_b'][i]
    idx = np.arange(64)
    f[idx, C_FOLD + idx] = 1.0
    f[64 + idx, C_FOLD + idx] = 1.0
    f[idx, C_EXP + idx] = 1.0
    f[idx, C_EXP + 64 + idx] = 1.0
    f[0:2, C_XW:C_XW + 64] = inp['v_lin0_w']
    f[64:66, C_XW:C_XW + 64] = inp['v_lin0_w']

    for i in range(L):
        bfc[:64, 128 * i:128 * i + 64] = inp['e_w'][i]
        bfc[64:, 128 * i + 64:128 * i + 128] = inp['e_w'][i]
    bfc[:64, 384] = inp['e_lin1_w'][:, 0]
    bfc[64:, 385] = inp['e_lin1_w'][:, 0]
    bfc[:, 386:514] = np.eye(128, dtype=np.float32)
    # embed broadcast stationary: K=2 rows (graph pair) -> 128 feature cols
    ew0 = inp['e_lin0_w'][0]
    for base in (0, 32):
        bfc[base, C_EMB:C_EMB + 64] = ew0
        bfc[base + 1, C_EMB + 64:C_EMB + 128] = ew0
    ws = [inp['v_w1'], inp['v_w2'], inp['v_w3'], inp['v_w4']]
    for i in range(L):
        for k in range(4):
            c = C_VWB + (i * 4 + k) * 64
            bfc[:64, c:c + 64] = ws[k][i]
            bfc[64:, c:c + 64] = ws[k][i]
    return f, bfc.astype(ml_dtypes.bfloat16)


def _diag_ap(t_ap, n_src, start=0):
    """AP over diag cols: start, start+201, ... (n_src entries), all 128 parts."""
    return bass.AP(t_ap.tensor, t_ap.offset + start,
                   [[t_ap.ap[0][0], 128], [N + 1, n_src]])


def build_nc():
    nc = bacc.Bacc(None, target_bir_lowering=False, debug=False,
                   num_devices=NCORES)
    x_d = nc.declare_dram_parameter("x", [BC, N, 2], F32, isOutput=False)
    adj_d = nc.declare_dram_parameter("adj", [BC, N, N], F32, isOutput=False)
    cf_d = nc.declare_dram_parameter("cf", [128, CF], F32, isOutput=False)
    cb_d = nc.declare_dram_parameter("cb", [128, CB], BF16, isOutput=False)
    out_d = nc.declare_dram_parameter("out", [BC, N, N], F32, isOutput=True)

    rg = [list(range(NCORES))]

    with tile.TileContext(nc) as tc:
        with (
            tc.tile_pool(name="big", bufs=1) as big,
            tc.tile_pool(name="sb", bufs=1) as sb,
            tc.tile_pool(name="scr", bufs=2) as scr,
            tc.tile_pool(name="ps", bufs=1, space="PSUM") as psp,
            tc.tile_pool(name="dram", bufs=1, space="DRAM") as dram,
        ):
            # ---------- persistent tiles ----------
            w_sb = big.tile([128, NCOL], BF16, tag="w")
            z_sb = big.tile([128, NCOL], BF16, tag="bigz")
            cf = sb.tile([128, CF], F32)
            cb = sb.tile([128, CB], BF16)
            nc.sync.dma_start(cf[:], cf_d[:])
            nc.sync.dma_start(cb[:], cb_d[:])

            ps = psp.tile([128, 4096], F32, tag="ps")

            h = sb.tile([128, N], F32)
            hb = sb.tile([128, N], BF16)
            x1f = sb.tile([128, N], F32)
            x2b = sb.tile([128, N], BF16)
            x3b = sb.tile([128, N], BF16)
            x4b = sb.tile([128, N], BF16)
            x4d = sb.tile([128, 2 * N], BF16)
            pooled = sb.tile([128, N], BF16)
            zv = sb.tile([128, N], F32)
            dtile = sb.tile([128, N], F32)
            ebn = sb.tile([128, 6 * (NCOL // ZCH)], F32)
            ebn6 = sb.tile([128, 6], F32)
            agg = sb.tile([128, 2], F32)
            stats_e = sb.tile([128, 6], F32)
            tmp2 = sb.tile([64, 2], F32)
            msq = sb.tile([64, 2], F32)
            var1 = sb.tile([64, 1], F32)
            sd1 = sb.tile([64, 1], F32)
            inv1 = sb.tile([64, 1], F32)
            prm = sb.tile([64, 2], F32)
            pe_sb = sb.tile([128, 2], F32)
            pv_sb = sb.tile([128, 2], F32)
            ar_sb = sb.tile([64, 6], F32)
            zer = sb.tile([2, N], F32)
            nc.vector.memset(zer[:], 0.0)

            def ccol(c, p0=0, p1=128):
                return cf[p0:p1, c:c + 1]

            def bank_ap(nbank, ncols, bank0=0):
                """PSUM read AP: nbank banks from bank0, ncols each."""
                return bass.AP(ps.tensor, ps[:].offset + bank0 * BANK,
                               [[ps[:].ap[0][0], 128], [BANK, nbank],
                                [1, ncols]])

            # ---------- init: h embed ----------
            xt = sb.tile([128, N], F32)
            nc.vector.memset(xt[:], 0.0)
            xr = x_d[:].rearrange("b n c -> b c n")
            nc.sync.dma_start(xt[0:2, :], xr[0])
            nc.sync.dma_start(xt[64:66, :], xr[1])
            nc.tensor.matmul(ps[0:64, 3584:3584 + N], cf[0:2, C_XW:C_XW + 64],
                             xt[0:2, :], start=True, stop=True)
            nc.tensor.matmul(ps[64:128, 3584:3584 + N], cf[64:66, C_XW:C_XW + 64],
                             xt[64:66, :], start=True, stop=True)
            nc.scalar.activation(h[:], ps[:, 3584:3584 + N], AF.Lrelu,
                                 bias=ccol(C_VB0), scale=1.0, alpha=0.01)
            nc.vector.tensor_copy(hb[:], h[:])

            # ---------- init: w embed ----------
            # adj rows: graph g half0 -> partition g, half1 -> partition 32+g
            adjf = big.tile([128, NCOL // 2], F32, tag="bigz")
            af = adj_d[:].rearrange("b u v -> b (u v)")
            half = NCOL // 2
            qq = half // 4
            for g in range(BC):
                for pc in range(4):
                    cs = slice(pc * qq, (pc + 1) * qq)
                    nc.sync.dma_start(adjf[g:g + 1, cs], af[g:g + 1, cs])
                    nc.scalar.dma_start(
                        adjf[32 + g:33 + g, cs],
                        af[g:g + 1, half + pc * qq:half + (pc + 1) * qq])
            for h2, base in ((0, 0), (1, 32)):
                for gi in range(half // EG):
                    adjbt = scr.tile([128, EG], BF16, tag="sg")
                    gcs = slice(gi * EG, (gi + 1) * EG)
                    nc.vector.tensor_copy(adjbt[base:base + 2, :],
                                          adjf[base:base + 2, gcs])
                    b0 = (gi % 2) * 4
                    for j in range(4):
                        nc.tensor.matmul(
                            ps[:, (b0 + j) * BANK:(b0 + j) * BANK + ECH],
                            cb[base:base + 2, C_EMB:C_EMB + 128],
                            adjbt[base:base + 2, j * ECH:(j + 1) * ECH],
                            start=True, stop=True)
                    wcs = h2 * half + gi * EG
                    nc.scalar.activation(
                        w_sb[:, wcs:wcs + EG].rearrange("p (c v) -> p c v",
                                                        v=ECH),
                        bank_ap(4, ECH, b0), AF.Lrelu, bias=ccol(C_EB0),
                        scale=1.0, alpha=0.01)
                    # zero this group's diagonal (per-group so layer-0 z
                    # matmuls can stream behind the embed)
                    u0 = (h2 * 10 + gi) * PCH_S
                    nc.vector.memset(
                        bass.AP(w_sb.tensor, w_sb[:].offset + u0 * (N + 1),
                                [[w_sb[:].ap[0][0], 128], [N + 1, PCH_S]]),
                        0.0)

            # ---------- helpers ----------
            def to_ssq(dst2, n_elem):
                """agg [128,2]=(mean,var) -> dst2 2 cols = (S, SS)."""
                nc.vector.tensor_tensor(dst2[:, 1:2], agg[:, 0:1], agg[:, 0:1],
                                        OP.mult)
                nc.vector.tensor_tensor(dst2[:, 1:2], dst2[:, 1:2], agg[:, 1:2],
                                        OP.add)
                nc.vector.tensor_scalar_mul(dst2[:, 1:2], dst2[:, 1:2],
                                            float(n_elem))
                nc.vector.tensor_scalar_mul(dst2[:, 0:1], agg[:, 0:1],
                                            float(n_elem))

            def bn_params(ar_ap, gcol, bcol, inv_cnt, out_sb):
                """ar_ap [64,2]=(S,SS) global -> out_sb [128,2]=(g', b')."""
                nc.vector.tensor_scalar_mul(msq[:], ar_ap, inv_cnt)
                nc.vector.tensor_tensor(var1[:], msq[:, 0:1], msq[:, 0:1], OP.mult)
                nc.vector.tensor_tensor(var1[:], msq[:, 1:2], var1[:], OP.subtract)
                nc.vector.tensor_scalar_add(var1[:], var1[:], EPS)
                nc.scalar.sqrt(sd1[:], var1[:])
                nc.vector.reciprocal(inv1[:], sd1[:])
                nc.vector.tensor_tensor(prm[:, 0:1], ccol(gcol, 0, 64), inv1[:],
                                        OP.mult)
                nc.vector.tensor_tensor(prm[:, 1:2], msq[:, 0:1], prm[:, 0:1],
                                        OP.mult)
                nc.vector.tensor_tensor(prm[:, 1:2], ccol(bcol, 0, 64),
                                        prm[:, 1:2], OP.subtract)
                nc.tensor.matmul(ps[:, 3072:3074], cf[0:64, C_EXP:C_EXP + 128],
                                 prm[:], start=True, stop=True)
                nc.scalar.copy(out_sb[:], ps[:, 3072:3074])

            def kick_ar(cols, tag):
                """AllReduce ar_sb[:, cols] across cores -> sb tile [64, n]."""
                n = cols.stop - cols.start
                ain = dram.tile([64, n], F32, tag=f"ai{tag}")
                aout = dram.tile([64, n], F32, tag=f"ao{tag}")
                nc.sync.dma_start(ain[:], ar_sb[:, cols])
                nc.gpsimd.collective_compute("AllReduce", OP.add,
                                             replica_groups=rg,
                                             ins=[ain.opt()],
                                             outs=[aout.opt()])
                res = sb.tile([64, n], F32, tag=f"ab{tag}")
                nc.gpsimd.dma_start(res[:], aout[:])
                return res

            def dummy_ar(tag):
                """Tiny collective to re-sync cores / keep CC warm; result
                unused. Pre-pays barrier skew while compute covers it."""
                ain = dram.tile([64, 1], F32, tag=f"di{tag}")
                aout = dram.tile([64, 1], F32, tag=f"do{tag}")
                nc.sync.dma_start(ain[:], ar_sb[:, 0:1])
                nc.gpsimd.collective_compute("AllReduce", OP.add,
                                             replica_groups=rg,
                                             ins=[ain.opt()],
                                             outs=[aout.opt()])

            def fold_stats(scol0, ncols):
                """stats_e[:, scol0:scol0+ncols] fold 128->64 into ar_sb."""
                nc.tensor.matmul(ps[0:64, 2048:2048 + ncols],
                                 cf[:, C_FOLD:C_FOLD + 64],
                                 stats_e[:, scol0:scol0 + ncols],
                                 start=True, stop=True)
                nc.scalar.copy(ar_sb[:, scol0:scol0 + ncols],
                               ps[0:64, 2048:2048 + ncols])

            # final-output: term DMA layout (pack = 4 OCH chunks = ACH cols)
            of = out_d[:].rearrange("b u v -> b (u v)")
            NPK = NCOL // ACH  # 20 packs
            t1_d = dram.tile([128, NPK * OCH], F32, tag="t1d")

            def pack_mms(p, src, bank):
                """e_lin1^T @ src for pack p into psum bank (rows 32j:32j+2)."""
                for j in range(4):
                    cols = slice(p * ACH + j * OCH, p * ACH + (j + 1) * OCH)
                    nc.tensor.matmul(
                        ps[32 * j:32 * j + 2, bank * BANK:bank * BANK + OCH],
                        cb[:, 384:386], src[:, cols], start=True, stop=True,
                        tile_position=(0, 32 * j))

            def out_dma(p, oc):
                pst = oc[:].ap[0][0]
                q = nc.gpsimd if p % 2 == 0 else nc.sync
                for g in range(BC):
                    src = bass.AP(oc.tensor, oc[:].offset + g * pst,
                                  [[32 * pst, 4], [1, OCH]])
                    dst = bass.AP(of.tensor, of.offset + g * NCOL + p * ACH,
                                  [[OCH, 4], [1, OCH]])
                    q.dma_start(dst, src)
                # zero this pack's output diagonal (same queue: ordered)
                dd = bass.AP(of.tensor, of.offset + p * ACH + PCH_S * p,
                             [[NCOL, 2], [N + 1, PCH_S]])
                q.dma_start(dd, zer[0:2, 0:PCH_S])

            # ---------- layers ----------
            stage = int(os.environ.get("KSTAGE", "3"))
            nlay = {0: 0, 1: 1, 2: 1, 3: L}[stage]
            pool_on = stage >= 2

            def is_last(i):
                return (i == L - 1) or not pool_on

            def x_block(i):
                """x1..x4 matmuls for layer i (bf16): banks 0-3 of psum."""
                for k, (dst, bcol) in enumerate(
                        ((x1f, C_B1 + i), (x2b, C_B2 + i),
                         (x3b, C_B3 + i), (x4b, C_B4 + i))):
                    if is_last(i) and k < 2:
                        continue
                    wc = C_VWB + (i * 4 + k) * 64
                    pxc = slice(k * BANK, k * BANK + N)
                    nc.tensor.matmul(ps[0:64, pxc], cb[0:64, wc:wc + 64],
                                     hb[0:64, :], start=True, stop=True)
                    nc.tensor.matmul(ps[64:128, pxc], cb[64:128, wc:wc + 64],
                                     hb[64:128, :], start=True, stop=True,
                                     tile_position=(64, 64))
                    nc.scalar.activation(dst[:], ps[:, pxc], AF.Identity,
                                         bias=ccol(bcol), scale=1.0)
                nc.vector.tensor_copy(x4d[:, 0:N], x4b[:])
                nc.vector.tensor_copy(x4d[:, N:2 * N], x4b[:])

            if nlay > 0:
                x_block(0)

            ASPL = 4  # apply chunks before the h-update/x-block handoff
            NPCH = NCOL // PCH  # 20 pool chunks

            for i in range(nlay):
                last = is_last(i)

                def pool_chunk(p):
                    cols = slice(p * PCH, (p + 1) * PCH)
                    u0 = p * PCH_S
                    sg = scr.tile([128, PCH], BF16, tag="sg")
                    nc.scalar.activation(sg[:], w_sb[:, cols], AF.Sigmoid)
                    pr = scr.tile([128, PCH], BF16, tag="pr")
                    r2 = x2b[:].unsqueeze(1).broadcast_to([128, PCH_S, N])
                    nc.vector.tensor_tensor(
                        pr[:].rearrange("p (u v) -> p u v", v=N),
                        sg[:].rearrange("p (u v) -> p u v", v=N), r2,
                        OP.mult)
                    nc.vector.memset(_diag_ap(pr[:], PCH_S, start=u0), -1e30)
                    # max tree: 200 -> 100 -> 50 -> reduce
                    po, pst = pr[:].offset, pr[:].ap[0][0]
                    ha = bass.AP(pr.tensor, po,
                                 [[pst, 128], [N, PCH_S], [1, 100]])
                    hb_ = bass.AP(pr.tensor, po + 100,
                                  [[pst, 128], [N, PCH_S], [1, 100]])
                    nc.vector.tensor_tensor(ha, ha, hb_, OP.max)
                    qa = bass.AP(pr.tensor, po,
                                 [[pst, 128], [N, PCH_S], [1, 50]])
                    qb = bass.AP(pr.tensor, po + 50,
                                 [[pst, 128], [N, PCH_S], [1, 50]])
                    nc.vector.tensor_tensor(qa, qa, qb, OP.max)
                    nc.vector.reduce_max(pooled[:, u0:u0 + PCH_S], qa,
                                         axis=mybir.AxisListType.X)

                # ---- z pass: PE burst, grouped ACT copies, DVE bn_stats,
                #      pool chunks interleaved (they only read w/x2) ----
                for g in range(NZG):
                    b0 = (g % 2) * 4
                    for j in range(4):
                        c = 4 * g + j
                        u0 = 2 * c
                        bk = (b0 + j) * BANK
                        pzc = slice(bk, bk + ZCH)
                        nc.tensor.matmul(ps[:, pzc], cb[:, 128 * i:128 * i + 128],
                                         w_sb[:, c * ZCH:(c + 1) * ZCH],
                                         start=True, stop=False)
                        pz3 = ps[:, pzc].rearrange("p (u v) -> p u v", v=N)
                        r3 = x3b[:, u0:u0 + 2].unsqueeze(2).broadcast_to(
                            [128, 2, N])
                        nc.tensor.matmul(pz3, cb[:, 386:514], r3,
                                         start=False, stop=False)
                        nc.tensor.matmul(ps[:, pzc], cb[:, 386:514],
                                         x4d[:], start=False, stop=True)
                    gcs = slice(g * ZG, (g + 1) * ZG)
                    nc.scalar.activation(
                        z_sb[:, gcs].rearrange("p (c v) -> p c v", v=ZCH),
                        bank_ap(4, ZCH, b0), AF.Identity, bias=0.0, scale=1.0)
                    for j in range(4):
                        c = 4 * g + j
                        nc.vector.bn_stats(ebn[:, 6 * c:6 * c + 6],
                                           z_sb[:, c * ZCH:(c + 1) * ZCH])
                    if g == 0 or (last and g == 20):
                        dummy_ar(f"z{i}g{g}")

                # ---- e-stats + AR kick ----
                nc.vector.bn_aggr(agg[:],
                                  ebn[:].rearrange("p (c s) -> p c s", s=3))
                to_ssq(stats_e[:, 0:2], NCOL)
                nc.vector.tensor_tensor(dtile[:], x3b[:], x4b[:], OP.add)
                nc.vector.bn_stats(ebn6[:], dtile[:])
                nc.vector.bn_aggr(agg[:],
                                  ebn6[:].rearrange("p (c s) -> p c s", s=3))
                to_ssq(stats_e[:, 2:4], N)
                fold_stats(0, 4)
                ar_e = kick_ar(slice(0, 4), f"e{i}")

                # ---- pooling (hides AR-e) + v-stats + AR-v ----
                if not last:
                    for p in range(NPCH):
                        pool_chunk(p)
                    nc.vector.tensor_tensor(zv[:], x1f[:], pooled[:], OP.add)
                    nc.vector.bn_stats(ebn6[:], zv[:])
                    nc.vector.bn_aggr(agg[:],
                                      ebn6[:].rearrange("p (c s) -> p c s", s=3))
                    to_ssq(stats_e[:, 4:6], N)
                    fold_stats(4, 2)
                    ar_v = kick_ar(slice(4, 6), f"v{i}")

                if last:
                    # term1 = e1^T.w_old staged to DRAM while AR-e flies
                    for p in range(NPK):
                        bk = p % 8
                        pack_mms(p, w_sb, bk)
                        oc = scr.tile([128, OCH], F32, tag="oc")
                        nc.scalar.activation(
                            oc[:], ps[:, bk * BANK:bk * BANK + OCH],
                            AF.Identity, bias=ccol(C_EL1B), scale=1.0)
                        nc.sync.dma_start(t1_d[:, p * OCH:(p + 1) * OCH],
                                          oc[:])

                # ---- e-BN params; apply to w ----
                nc.vector.tensor_tensor(tmp2[:], ar_e[:, 0:2], ar_e[:, 2:4],
                                        OP.subtract)
                bn_params(tmp2[:], C_EG + i, C_EB + i, 1.0 / CNT_E, pe_sb)
                if not last:
                    def apply_chunk(a):
                        cols = slice(a * ACH, (a + 1) * ACH)
                        lr = scr.tile([128, ACH], BF16, tag="lr")
                        nc.scalar.activation(lr[:], z_sb[:, cols], AF.Lrelu,
                                             bias=pe_sb[:, 1:2],
                                             scale=pe_sb[:, 0:1], alpha=0.01)
                        nc.vector.tensor_tensor(w_sb[:, cols], w_sb[:, cols],
                                                lr[:], OP.add)
                        nc.vector.memset(
                            bass.AP(w_sb.tensor,
                                    w_sb[:].offset + a * ACH + PCH_S * a,
                                    [[w_sb[:].ap[0][0], 128],
                                     [N + 1, PCH_S]]), 0.0)

                    for a in range(ASPL):
                        apply_chunk(a)
                    # h update (needs AR-v) + next layer's x-block, emitted
                    # mid-apply so the next z-burst streams behind the apply
                    bn_params(ar_v[:], C_VG + i, C_VB + i, 1.0 / CNT_V, pv_sb)
                    hup = sb.tile([128, N], F32, tag="hup")
                    nc.scalar.activation(hup[:], zv[:], AF.Lrelu,
                                         bias=pv_sb[:, 1:2],
                                         scale=pv_sb[:, 0:1], alpha=0.01)
                    nc.vector.tensor_tensor(h[:], h[:], hup[:], OP.add)
                    nc.vector.tensor_copy(hb[:], h[:])
                    x_block(i + 1)
                    for a in range(ASPL, NCOL // ACH):
                        apply_chunk(a)
                else:
                    # final: lr chunks; term2 accumulated and summed
                    for p in range(NPK):
                        cols = slice(p * ACH, (p + 1) * ACH)
                        lr = scr.tile([128, ACH], BF16, tag="lr")
                        nc.scalar.activation(lr[:], z_sb[:, cols], AF.Lrelu,
                                             bias=pe_sb[:, 1:2],
                                             scale=pe_sb[:, 0:1], alpha=0.01)
                        bk = p % 8
                        for j in range(4):
                            nc.tensor.matmul(
                                ps[32 * j:32 * j + 2,
                                   bk * BANK:bk * BANK + OCH],
                                cb[:, 384:386], lr[:, j * OCH:(j + 1) * OCH],
                                start=True, stop=True,
                                tile_position=(0, 32 * j))
                        t1b = scr.tile([128, OCH], F32, tag="t1b")
                        nc.sync.dma_start(t1b[:],
                                          t1_d[:, p * OCH:(p + 1) * OCH])
                        oc = scr.tile([128, OCH], F32, tag="oc")
                        nc.vector.scalar_tensor_tensor(
                            oc[:], ps[:, bk * BANK:bk * BANK + OCH], 0.0,
                            t1b[:], OP.add, OP.add)
                        out_dma(p, oc)
    nc.compile()
    return nc


def _get_nc():
    if "nc" not in _CACHE:
        _CACHE["nc"] = build_nc()
    return _CACHE["nc"]


def run(inputs, **kw):
    inp = {k: np.asarray(v, np.float32) for k, v in inputs.items()}
    cfh, cbh = _build_consts(inp)
    nc = _get_nc()
    in_maps = []
    for c in range(NCORES):
        sl = slice(c * BC, (c + 1) * BC)
        in_maps.append({
            "x": np.ascontiguousarray(inp['x'][sl]),
            "adj": np.ascontiguousarray(inp['adj'][sl]),
            "cf": cfh, "cb": cbh,
        })
    res = run_bass_kernel_spmd(nc, in_maps, core_ids=list(range(NCORES)), **kw)
    out = np.concatenate([res.results[c]["out"] for c in range(NCORES)], axis=0)
    return out, res


def kernel(**inputs) -> np.ndarray:
    out, _ = run(inputs)
    return out


# revision 31
# speedup vs baseline: 1.0126x; 1.0126x over previous
"""Trainium2 Bass kernel for nn_DIMESDenseEncoder (GNN message passing).

Self-contained: hardcodes B=16, N=200, U=64, L=3, 8 cores, batch-sharded
(2 graphs per core). Dense edge layout [src*200+dst] with masked diagonal;
feature-major SBUF layout: partition = graph_half*64 + feature.
BatchNorm stats are exact: per-core partial sums, cross-core AllReduce,
analytic diagonal correction. Edge tensors (w, z) live in SBUF as bf16.

v2: bf16 embed broadcast (one matmul per chunk, K=2), manual 8-bank PSUM
rotation with 4-bank grouped ACT copies (+S accumulator), SS via DVE
scalar_tensor_tensor at 2x, split AllReduce (edge stats hidden behind
pooling, node stats behind apply), bf16 x-matmuls, last-layer final output
split as e1^T.w_old + e1^T.lr with term1 staged via DRAM during the AR gap.
"""
import os
import numpy as np
import ml_dtypes
import concourse.bass as bass
import concourse.tile as tile
from concourse import bacc, mybir
from concourse.bass_utils import run_bass_kernel_spmd

F32, BF16 = mybir.dt.float32, mybir.dt.bfloat16
F16 = mybir.dt.float16
AF = mybir.ActivationFunctionType
OP = mybir.AluOpType

B, N, U, L = 16, 200, 64, 3
EPS = 1e-5
NCORES = 8
BC = B // NCORES            # graphs per core
NCOL = N * N                # dense edge cols per graph-half = 40000
CNT_E = B * N * (N - 1)     # global real-edge count
CNT_V = B * N               # global node count

ZCH = 400                   # z chunk = 2 sources, one PSUM bank
ZG = 4 * ZCH                # z copy group = 4 banks = 1600 cols
NZG = NCOL // ZG            # 25 groups per layer
PCH_S = 10                  # pooling chunk sources
PCH = PCH_S * N             # pooling chunk cols = 2000
ACH = 2000                  # apply chunk cols (10 sources)
OCH = 500                   # final-output sub-chunk
ECH = 500                   # embed matmul sub-chunk (one bank)
EG = 4 * ECH                # embed group cols = 2000
BANK = 512                  # PSUM bank stride in f32 cols

# ---- consts_f32 column layout (host-packed) ----
C_VB0 = 0        # v_lin0_b stacked
C_EB0 = 1        # e_lin0_b stacked
C_EL1B = 2       # e_lin1_b at p {0,1,32,33,64,65,96,97}
C_B1 = 3         # v_b1[i] stacked (3 cols)
C_B2 = 6
C_B3 = 9         # v_b3[i]+e_b[i] stacked
C_B4 = 12
C_EG = 15        # e_bn_g[i] p0-63
C_EB = 18
C_VG = 21
C_VB = 24
C_FOLD = 27      # P_fold [128,64]
C_EXP = 91       # E_exp [64,128]
C_XW = 219       # v_lin0_w rows 0-1 / 64-65
CF = C_XW + 64

# ---- consts_bf16 columns ----
# 128*i : e_w[i] blockdiag [128,128]; 384:386 e_lin1 blockdiag;
# 386:514 I128; 514:642 embed stationary (rows 0,1,32,33);
# 642: v_wk[i] stacked-two-copies, 12 blocks of 64
C_EMB = 514
C_VWB = 642
CB = C_VWB + 12 * 64

_CACHE = {}


def _build_consts(inp):
    f = np.zeros((128, CF), np.float32)
    bfc = np.zeros((128, CB), np.float32)

    def stack(v):
        return np.concatenate([v, v]).astype(np.float32)

    f[:, C_VB0] = stack(inp['v_lin0_b'])
    f[:, C_EB0] = stack(inp['e_lin0_b'])
    for _q in (0, 32, 64, 96):
        f[_q, C_EL1B] = f[_q + 1, C_EL1B] = inp['e_lin1_b'][0]
    for i in range(L):
        f[:, C_B1 + i] = stack(inp['v_b1'][i])
        f[:, C_B2 + i] = stack(inp['v_b2'][i])
        f[:, C_B3 + i] = stack(inp['v_b3'][i] + inp['e_b'][i])
        f[:, C_B4 + i] = stack(inp['v_b4'][i])
        f[:64, C_EG + i] = inp['e_bn_g'][i]
        f[:64, C_EB + i] = inp['e_bn_b'][i]
        f[:64, C_VG + i] = inp['v_bn_g'][i]
        f[:64, C_VB + i] = inp['v_bn_b'][i]
    idx = np.arange(64)
    f[idx, C_FOLD + idx] = 1.0
    f[64 + idx, C_FOLD + idx] = 1.0
    f[idx, C_EXP + idx] = 1.0
    f[idx, C_EXP + 64 + idx] = 1.0
    f[0:2, C_XW:C_XW + 64] = inp['v_lin0_w']
    f[64:66, C_XW:C_XW + 64] = inp['v_lin0_w']

    for i in range(L):
        bfc[:64, 128 * i:128 * i + 64] = inp['e_w'][i]
        bfc[64:, 128 * i + 64:128 * i + 128] = inp['e_w'][i]
    bfc[:64, 384] = inp['e_lin1_w'][:, 0]
    bfc[64:, 385] = inp['e_lin1_w'][:, 0]
    bfc[:, 386:514] = np.eye(128, dtype=np.float32)
    # embed broadcast stationary: K=2 rows (graph pair) -> 128 feature cols
    ew0 = inp['e_lin0_w'][0]
    for base in (0, 32):
        bfc[base, C_EMB:C_EMB + 64] = ew0
        bfc[base + 1, C_EMB + 64:C_EMB + 128] = ew0
    ws = [inp['v_w1'], inp['v_w2'], inp['v_w3'], inp['v_w4']]
    for i in range(L):
        for k in range(4):
            c = C_VWB + (i * 4 + k) * 64
            bfc[:64, c:c + 64] = ws[k][i]
            bfc[64:, c:c + 64] = ws[k][i]
    return f, bfc.astype(ml_dtypes.bfloat16)


def _diag_ap(t_ap, n_src, start=0):
    """AP over diag cols: start, start+201, ... (n_src entries), all 128 parts."""
    return bass.AP(t_ap.tensor, t_ap.offset + start,
                   [[t_ap.ap[0][0], 128], [N + 1, n_src]])


def build_nc():
    nc = bacc.Bacc(None, target_bir_lowering=False, debug=False,
                   num_devices=NCORES)
    x_d = nc.declare_dram_parameter("x", [BC, N, 2], F32, isOutput=False)
    adj_d = nc.declare_dram_parameter("adj", [BC, N, N], F32, isOutput=False)
    cf_d = nc.declare_dram_parameter("cf", [128, CF], F32, isOutput=False)
    cb_d = nc.declare_dram_parameter("cb", [128, CB], BF16, isOutput=False)
    out_d = nc.declare_dram_parameter("out", [BC, N, N], F32, isOutput=True)

    rg = [list(range(NCORES))]

    with tile.TileContext(nc) as tc:
        with (
            tc.tile_pool(name="big", bufs=1) as big,
            tc.tile_pool(name="sb", bufs=1) as sb,
            tc.tile_pool(name="scr", bufs=2) as scr,
            tc.tile_pool(name="ps", bufs=1, space="PSUM") as psp,
            tc.tile_pool(name="dram", bufs=1, space="DRAM") as dram,
        ):
            # ---------- persistent tiles ----------
            w_sb = big.tile([128, NCOL], BF16, tag="w")
            z_sb = big.tile([128, NCOL], BF16, tag="bigz")
            cf = sb.tile([128, CF], F32)
            cb = sb.tile([128, CB], BF16)
            nc.sync.dma_start(cf[:], cf_d[:])
            nc.sync.dma_start(cb[:], cb_d[:])

            ps = psp.tile([128, 4096], F32, tag="ps")

            h = sb.tile([128, N], F32)
            hb = sb.tile([128, N], BF16)
            x1f = sb.tile([128, N], F32)
            x2b = sb.tile([128, N], BF16)
            x3b = sb.tile([128, N], BF16)
            x4b = sb.tile([128, N], BF16)
            x4d = sb.tile([128, 2 * N], BF16)
            pooled = sb.tile([128, N], BF16)
            zv = sb.tile([128, N], F32)
            dtile = sb.tile([128, N], F32)
            ebn = sb.tile([128, 6 * (NCOL // ZCH)], F32)
            ebn6 = sb.tile([128, 6], F32)
            agg = sb.tile([128, 2], F32)
            stats_e = sb.tile([128, 6], F32)
            tmp2 = sb.tile([64, 2], F32)
            msq = sb.tile([64, 2], F32)
            var1 = sb.tile([64, 1], F32)
            sd1 = sb.tile([64, 1], F32)
            inv1 = sb.tile([64, 1], F32)
            prm = sb.tile([64, 2], F32)
            pe_sb = sb.tile([128, 2], F32)
            pv_sb = sb.tile([128, 2], F32)
            ar_sb = sb.tile([64, 6], F32)
            zer = sb.tile([2, N], F32)
            nc.vector.memset(zer[:], 0.0)

            def ccol(c, p0=0, p1=128):
                return cf[p0:p1, c:c + 1]

            def bank_ap(nbank, ncols, bank0=0):
                """PSUM read AP: nbank banks from bank0, ncols each."""
                return bass.AP(ps.tensor, ps[:].offset + bank0 * BANK,
                               [[ps[:].ap[0][0], 128], [BANK, nbank],
                                [1, ncols]])

            # ---------- init: h embed ----------
            xt = sb.tile([128, N], F32)
            nc.vector.memset(xt[:], 0.0)
            xr = x_d[:].rearrange("b n c -> b c n")
            nc.sync.dma_start(xt[0:2, :], xr[0])
            nc.sync.dma_start(xt[64:66, :], xr[1])
            nc.tensor.matmul(ps[0:64, 3584:3584 + N], cf[0:2, C_XW:C_XW + 64],
                             xt[0:2, :], start=True, stop=True)
            nc.tensor.matmul(ps[64:128, 3584:3584 + N], cf[64:66, C_XW:C_XW + 64],
                             xt[64:66, :], start=True, stop=True)
            nc.scalar.activation(h[:], ps[:, 3584:3584 + N], AF.Lrelu,
                                 bias=ccol(C_VB0), scale=1.0, alpha=0.01)
            nc.vector.tensor_copy(hb[:], h[:])

            # ---------- init: w embed ----------
            # adj rows: graph g half0 -> partition g, half1 -> partition 32+g
            adjf = big.tile([128, NCOL // 2], F32, tag="bigz")
            af = adj_d[:].rearrange("b u v -> b (u v)")
            half = NCOL // 2
            qq = half // 4
            for g in range(BC):
                for pc in range(4):
                    cs = slice(pc * qq, (pc + 1) * qq)
                    nc.sync.dma_start(adjf[g:g + 1, cs], af[g:g + 1, cs])
                    nc.scalar.dma_start(
                        adjf[32 + g:33 + g, cs],
                        af[g:g + 1, half + pc * qq:half + (pc + 1) * qq])
            for h2, base in ((0, 0), (1, 32)):
                for gi in range(half // EG):
                    adjbt = scr.tile([128, EG], BF16, tag="sg")
                    gcs = slice(gi * EG, (gi + 1) * EG)
                    nc.vector.tensor_copy(adjbt[base:base + 2, :],
                                          adjf[base:base + 2, gcs])
                    b0 = (gi % 2) * 4
                    for j in range(4):
                        nc.tensor.matmul(
                            ps[:, (b0 + j) * BANK:(b0 + j) * BANK + ECH],
                            cb[base:base + 2, C_EMB:C_EMB + 128],
                            adjbt[base:base + 2, j * ECH:(j + 1) * ECH],
                            start=True, stop=True)
                    wcs = h2 * half + gi * EG
                    nc.scalar.activation(
                        w_sb[:, wcs:wcs + EG].rearrange("p (c v) -> p c v",
                                                        v=ECH),
                        bank_ap(4, ECH, b0), AF.Lrelu, bias=ccol(C_EB0),
                        scale=1.0, alpha=0.01)
                    # zero this group's diagonal (per-group so layer-0 z
                    # matmuls can stream behind the embed)
                    u0 = (h2 * 10 + gi) * PCH_S
                    nc.vector.memset(
                        bass.AP(w_sb.tensor, w_sb[:].offset + u0 * (N + 1),
                                [[w_sb[:].ap[0][0], 128], [N + 1, PCH_S]]),
                        0.0)

            # ---------- helpers ----------
            def to_ssq(dst2, n_elem):
                """agg [128,2]=(mean,var) -> dst2 2 cols = (S, SS)."""
                nc.vector.tensor_tensor(dst2[:, 1:2], agg[:, 0:1], agg[:, 0:1],
                                        OP.mult)
                nc.vector.tensor_tensor(dst2[:, 1:2], dst2[:, 1:2], agg[:, 1:2],
                                        OP.add)
                nc.vector.tensor_scalar_mul(dst2[:, 1:2], dst2[:, 1:2],
                                            float(n_elem))
                nc.vector.tensor_scalar_mul(dst2[:, 0:1], agg[:, 0:1],
                                            float(n_elem))

            def bn_params(ar_ap, gcol, bcol, inv_cnt, out_sb):
                """ar_ap [64,2]=(S,SS) global -> out_sb [128,2]=(g', b')."""
                nc.vector.tensor_scalar_mul(msq[:], ar_ap, inv_cnt)
                nc.vector.tensor_tensor(var1[:], msq[:, 0:1], msq[:, 0:1], OP.mult)
                nc.vector.tensor_tensor(var1[:], msq[:, 1:2], var1[:], OP.subtract)
                nc.vector.tensor_scalar_add(var1[:], var1[:], EPS)
                nc.scalar.sqrt(sd1[:], var1[:])
                nc.vector.reciprocal(inv1[:], sd1[:])
                nc.vector.tensor_tensor(prm[:, 0:1], ccol(gcol, 0, 64), inv1[:],
                                        OP.mult)
                nc.vector.tensor_tensor(prm[:, 1:2], msq[:, 0:1], prm[:, 0:1],
                                        OP.mult)
                nc.vector.tensor_tensor(prm[:, 1:2], ccol(bcol, 0, 64),
                                        prm[:, 1:2], OP.subtract)
                nc.tensor.matmul(ps[:, 3072:3074], cf[0:64, C_EXP:C_EXP + 128],
                                 prm[:], start=True, stop=True)
                nc.scalar.copy(out_sb[:], ps[:, 3072:3074])

            def kick_ar(cols, tag):
                """AllReduce ar_sb[:, cols] across cores -> sb tile [64, n]."""
                n = cols.stop - cols.start
                ain = dram.tile([64, n], F32, tag=f"ai{tag}")
                aout = dram.tile([64, n], F32, tag=f"ao{tag}")
                nc.sync.dma_start(ain[:], ar_sb[:, cols])
                nc.gpsimd.collective_compute("AllReduce", OP.add,
                                             replica_groups=rg,
                                             ins=[ain.opt()],
                                             outs=[aout.opt()])
                res = sb.tile([64, n], F32, tag=f"ab{tag}")
                nc.gpsimd.dma_start(res[:], aout[:])
                return res

            def dummy_ar(tag):
                """Tiny collective to re-sync cores / keep CC warm; result
                unused. Pre-pays barrier skew while compute covers it."""
                ain = dram.tile([64, 1], F32, tag=f"di{tag}")
                aout = dram.tile([64, 1], F32, tag=f"do{tag}")
                nc.sync.dma_start(ain[:], ar_sb[:, 0:1])
                nc.gpsimd.collective_compute("AllReduce", OP.add,
                                             replica_groups=rg,
                                             ins=[ain.opt()],
                                             outs=[aout.opt()])

            def fold_stats(scol0, ncols):
                """stats_e[:, scol0:scol0+ncols] fold 128->64 into ar_sb."""
                nc.tensor.matmul(ps[0:64, 2048:2048 + ncols],
                                 cf[:, C_FOLD:C_FOLD + 64],
                                 stats_e[:, scol0:scol0 + ncols],
                                 start=True, stop=True)
                nc.scalar.copy(ar_sb[:, scol0:scol0 + ncols],
                               ps[0:64, 2048:2048 + ncols])

            # final-output: term DMA layout (pack = 4 OCH chunks = ACH cols)
            of = out_d[:].rearrange("b u v -> b (u v)")
            NPK = NCOL // ACH  # 20 packs
            t1_d = dram.tile([128, NPK * OCH], F32, tag="t1d")

            def pack_mms(p, src, bank):
                """e_lin1^T @ src for pack p into psum bank (rows 32j:32j+2)."""
                for j in range(4):
                    cols = slice(p * ACH + j * OCH, p * ACH + (j + 1) * OCH)
                    nc.tensor.matmul(
                        ps[32 * j:32 * j + 2, bank * BANK:bank * BANK + OCH],
                        cb[:, 384:386], src[:, cols], start=True, stop=True,
                        tile_position=(0, 32 * j))

            def out_dma(p, oc):
                pst = oc[:].ap[0][0]
                q = nc.gpsimd if p % 2 == 0 else nc.sync
                for g in range(BC):
                    src = bass.AP(oc.tensor, oc[:].offset + g * pst,
                                  [[32 * pst, 4], [1, OCH]])
                    dst = bass.AP(of.tensor, of.offset + g * NCOL + p * ACH,
                                  [[OCH, 4], [1, OCH]])
                    q.dma_start(dst, src)
                # zero this pack's output diagonal (same queue: ordered)
                dd = bass.AP(of.tensor, of.offset + p * ACH + PCH_S * p,
                             [[NCOL, 2], [N + 1, PCH_S]])
                q.dma_start(dd, zer[0:2, 0:PCH_S])

            # ---------- layers ----------
            stage = int(os.environ.get("KSTAGE", "3"))
            nlay = {0: 0, 1: 1, 2: 1, 3: L}[stage]
            pool_on = stage >= 2

            def is_last(i):
                return (i == L - 1) or not pool_on

            def x_block(i):
                """x1..x4 matmuls for layer i (bf16): banks 0-3 of psum."""
                for k, (dst, bcol) in enumerate(
                        ((x1f, C_B1 + i), (x2b, C_B2 + i),
                         (x3b, C_B3 + i), (x4b, C_B4 + i))):
                    if is_last(i) and k < 2:
                        continue
                    wc = C_VWB + (i * 4 + k) * 64
                    pxc = slice(k * BANK, k * BANK + N)
                    nc.tensor.matmul(ps[0:64, pxc], cb[0:64, wc:wc + 64],
                                     hb[0:64, :], start=True, stop=True)
                    nc.tensor.matmul(ps[64:128, pxc], cb[64:128, wc:wc + 64],
                                     hb[64:128, :], start=True, stop=True,
                                     tile_position=(64, 64))
                    nc.scalar.activation(dst[:], ps[:, pxc], AF.Identity,
                                         bias=ccol(bcol), scale=1.0)
                nc.vector.tensor_copy(x4d[:, 0:N], x4b[:])
                nc.vector.tensor_copy(x4d[:, N:2 * N], x4b[:])

            if nlay > 0:
                x_block(0)

            ASPL = 4  # apply chunks before the h-update/x-block handoff
            NPCH = NCOL // PCH  # 20 pool chunks

            for i in range(nlay):
                last = is_last(i)

                def pool_chunk(p):
                    cols = slice(p * PCH, (p + 1) * PCH)
                    u0 = p * PCH_S
                    sg = scr.tile([128, PCH], BF16, tag="sg")
                    nc.scalar.activation(sg[:], w_sb[:, cols], AF.Sigmoid)
                    pr = scr.tile([128, PCH], BF16, tag="pr")
                    r2 = x2b[:].unsqueeze(1).broadcast_to([128, PCH_S, N])
                    nc.vector.tensor_tensor(
                        pr[:].rearrange("p (u v) -> p u v", v=N),
                        sg[:].rearrange("p (u v) -> p u v", v=N), r2,
                        OP.mult)
                    nc.vector.memset(_diag_ap(pr[:], PCH_S, start=u0), -1e30)
                    # max tree: 200 -> 100 -> 50 -> reduce
                    po, pst = pr[:].offset, pr[:].ap[0][0]
                    ha = bass.AP(pr.tensor, po,
                                 [[pst, 128], [N, PCH_S], [1, 100]])
                    hb_ = bass.AP(pr.tensor, po + 100,
                                  [[pst, 128], [N, PCH_S], [1, 100]])
                    nc.vector.tensor_tensor(ha, ha, hb_, OP.max)
                    qa = bass.AP(pr.tensor, po,
                                 [[pst, 128], [N, PCH_S], [1, 50]])
                    qb = bass.AP(pr.tensor, po + 50,
                                 [[pst, 128], [N, PCH_S], [1, 50]])
                    nc.vector.tensor_tensor(qa, qa, qb, OP.max)
                    nc.vector.reduce_max(pooled[:, u0:u0 + PCH_S], qa,
                                         axis=mybir.AxisListType.X)

                # ---- z pass: PE burst, grouped ACT copies, DVE bn_stats,
                #      pool chunks interleaved (they only read w/x2) ----
                for g in range(NZG):
                    b0 = (g % 2) * 4
                    for j in range(4):
                        c = 4 * g + j
                        u0 = 2 * c
                        bk = (b0 + j) * BANK
                        pzc = slice(bk, bk + ZCH)
                        nc.tensor.matmul(ps[:, pzc], cb[:, 128 * i:128 * i + 128],
                                         w_sb[:, c * ZCH:(c + 1) * ZCH],
                                         start=True, stop=False)
                        pz3 = ps[:, pzc].rearrange("p (u v) -> p u v", v=N)
                        r3 = x3b[:, u0:u0 + 2].unsqueeze(2).broadcast_to(
                            [128, 2, N])
                        nc.tensor.matmul(pz3, cb[:, 386:514], r3,
                                         start=False, stop=False)
                        nc.tensor.matmul(ps[:, pzc], cb[:, 386:514],
                                         x4d[:], start=False, stop=True)
                    gcs = slice(g * ZG, (g + 1) * ZG)
                    nc.scalar.activation(
                        z_sb[:, gcs].rearrange("p (c v) -> p c v", v=ZCH),
                        bank_ap(4, ZCH, b0), AF.Identity, bias=0.0, scale=1.0)
                    for j in range(4):
                        c = 4 * g + j
                        nc.vector.bn_stats(ebn[:, 6 * c:6 * c + 6],
                                           z_sb[:, c * ZCH:(c + 1) * ZCH])
                    if g == 0:
                        dummy_ar(f"z{i}g{g}")

                # ---- e-stats + AR kick ----
                nc.vector.bn_aggr(agg[:],
                                  ebn[:].rearrange("p (c s) -> p c s", s=3))
                to_ssq(stats_e[:, 0:2], NCOL)
                nc.vector.tensor_tensor(dtile[:], x3b[:], x4b[:], OP.add)
                nc.vector.bn_stats(ebn6[:], dtile[:])
                nc.vector.bn_aggr(agg[:],
                                  ebn6[:].rearrange("p (c s) -> p c s", s=3))
                to_ssq(stats_e[:, 2:4], N)
                fold_stats(0, 4)
                ar_e = kick_ar(slice(0, 4), f"e{i}")

                # ---- pooling (hides AR-e) + v-stats + AR-v ----
                if not last:
                    for p in range(NPCH):
                        pool_chunk(p)
                    nc.vector.tensor_tensor(zv[:], x1f[:], pooled[:], OP.add)
                    nc.vector.bn_stats(ebn6[:], zv[:])
                    nc.vector.bn_aggr(agg[:],
                                      ebn6[:].rearrange("p (c s) -> p c s", s=3))
                    to_ssq(stats_e[:, 4:6], N)
                    fold_stats(4, 2)
                    ar_v = kick_ar(slice(4, 6), f"v{i}")

                if last:
                    # term1 = e1^T.w_old staged to DRAM while AR-e flies
                    for p in range(NPK):
                        bk = p % 8
                        pack_mms(p, w_sb, bk)
                        oc = scr.tile([128, OCH], F32, tag="oc")
                        nc.scalar.activation(
                            oc[:], ps[:, bk * BANK:bk * BANK + OCH],
                            AF.Identity, bias=ccol(C_EL1B), scale=1.0)
                        nc.sync.dma_start(t1_d[:, p * OCH:(p + 1) * OCH],
                                          oc[:])

                # ---- e-BN params; apply to w ----
                nc.vector.tensor_tensor(tmp2[:], ar_e[:, 0:2], ar_e[:, 2:4],
                                        OP.subtract)
                bn_params(tmp2[:], C_EG + i, C_EB + i, 1.0 / CNT_E, pe_sb)
                if not last:
                    def apply_chunk(a):
                        cols = slice(a * ACH, (a + 1) * ACH)
                        lr = scr.tile([128, ACH], BF16, tag="lr")
                        nc.scalar.activation(lr[:], z_sb[:, cols], AF.Lrelu,
                                             bias=pe_sb[:, 1:2],
                                             scale=pe_sb[:, 0:1], alpha=0.01)
                        nc.vector.tensor_tensor(w_sb[:, cols], w_sb[:, cols],
                                                lr[:], OP.add)
                        nc.vector.memset(
                            bass.AP(w_sb.tensor,
                                    w_sb[:].offset + a * ACH + PCH_S * a,
                                    [[w_sb[:].ap[0][0], 128],
                                     [N + 1, PCH_S]]), 0.0)

                    for a in range(ASPL):
                        apply_chunk(a)
                    # h update (needs AR-v) + next layer's x-block, emitted
                    # mid-apply so the next z-burst streams behind the apply
                    bn_params(ar_v[:], C_VG + i, C_VB + i, 1.0 / CNT_V, pv_sb)
                    hup = sb.tile([128, N], F32, tag="hup")
                    nc.scalar.activation(hup[:], zv[:], AF.Lrelu,
                                         bias=pv_sb[:, 1:2],
                                         scale=pv_sb[:, 0:1], alpha=0.01)
                    nc.vector.tensor_tensor(h[:], h[:], hup[:], OP.add)
                    nc.vector.tensor_copy(hb[:], h[:])
                    x_block(i + 1)
                    for a in range(ASPL, NCOL // ACH):
                        apply_chunk(a)
                else:
                    # final: lr chunks; term2 accumulated and summed
                    for p in range(NPK):
                        cols = slice(p * ACH, (p + 1) * ACH)
                        lr = scr.tile([128, ACH], BF16, tag="lr")
                        nc.scalar.activation(lr[:], z_sb[:, cols], AF.Lrelu,
                                             bias=pe_sb[:, 1:2],
                                             scale=pe_sb[:, 0:1], alpha=0.01)
                        bk = p % 8
                        for j in range(4):
                            nc.tensor.matmul(
                                ps[32 * j:32 * j + 2,
                                   bk * BANK:bk * BANK + OCH],
                                cb[:, 384:386], lr[:, j * OCH:(j + 1) * OCH],
                                start=True, stop=True,
                                tile_position=(0, 32 * j))
                        t1b = scr.tile([128, OCH], F32, tag="t1b")
                        nc.sync.dma_start(t1b[:],
                                          t1_d[:, p * OCH:(p + 1) * OCH])
                        oc = scr.tile([128, OCH], F32, tag="oc")
                        nc.vector.scalar_tensor_tensor(
                            oc[:], ps[:, bk * BANK:bk * BANK + OCH], 0.0,
                            t1b[:], OP.add, OP.add)
                        out_dma(p, oc)
    nc.compile()
    return nc


def _get_nc():
    if "nc" not in _CACHE:
        _CACHE["nc"] = build_nc()
    return _CACHE["nc"]


def run(inputs, **kw):
    inp = {k: np.asarray(v, np.float32) for k, v in inputs.items()}
    cfh, cbh = _build_consts(inp)
    nc = _get_nc()
    in_maps = []
    for c in range(NCORES):
        sl = slice(c * BC, (c + 1) * BC)
        in_maps.append({
            "x": np.ascontiguousarray(inp['x'][sl]),
            "adj": np.ascontiguousarray(inp['adj'][sl]),
            "cf": cfh, "cb": cbh,
        })
    res = run_bass_kernel_spmd(nc, in_maps, core_ids=list(range(NCORES)), **kw)
    out = np.concatenate([res.results[c]["out"] for c in range(NCORES)], axis=0)
    return out, res


def kernel(**inputs) -> np.ndarray:
    out, _ = run(inputs)
    return out
